# revision 39
# baseline (speedup 1.0000x reference)
"""Trainium2 Bass kernel for nn_DRCapsuleFC_79817672229002 (8 NeuronCores).

Math (see reference):
  B=128, N=2048, A=16, M=64, D=16; qk = 1/M (uniform routing)
  v[b,m,d]    = (1/M) * sum_{n,a} x[b,n,a] * w[n,a,m,d]
  vs          = squash_d(v) = v * sqrt(|v|^2)/(1+|v|^2)
  new_qk[b,n,m] = sum_{a,d} x[b,n,a] * w[n,a,m,d] * vs[b,m,d]
  v_out       = LN_d(vs) * gamma + beta

Sharding: tensor-parallel over n (in_n_capsules): 8 cores x 256 n each.
Per core:
  - host prepares XTe/XTo = x shard transposed to [na, b] (bf16) with
    odd/even n_local rows zeroed (enables K=32 row-tile matmuls that
    compute per-capsule U_n without mixing neighbouring capsules).
  - phase 1: v_part = sum_j (XTe_j + XTo_j).T @ W_j  (w streamed, bf16)
    -> fp32 on-chip AllReduce (512KB) -> squash + LN (DVE/ACT)
  - phase 2: w streamed again; per 128-row chunk, parity and md-half,
    4 concurrent K=32 row-tile matmuls produce U_n[b, md] in PSUM;
    ScalarE drains PSUM->SBUF bf16, DVE multiplies by vs, DVE/GpSimd
    tree-reduce over d -> new_qk[b,n,m].

All matmuls run in bf16 (fp32 matmul is quarter-rate on the PE);
accumulation stays fp32 in PSUM. Walrus in this container accepts at
most ONE sync-wait per instruction, so _split_waits() spreads Tile's
multi-waits over same-engine NOPs after scheduling.
"""

import sys

sys.path.insert(0, "/opt/trn_rl_repo")

import numpy as np

import concourse.bass as bass
import concourse.mybir as mybir
import concourse.tile as tile
from concourse.bass_utils import run_bass_kernel_spmd

F32 = mybir.dt.float32
BF16 = mybir.dt.bfloat16

B = 128
N = 2048
A = 16
M = 64
D = 16
MD = M * D  # 1024
NCORES = 8
NS = N // NCORES  # 256
KS = NS * A  # 4096
NCHUNK = KS // 128  # 32
LN_EPS = 1e-5


def _split_waits(nc, limit=1):
    """Walrus in this container accepts at most one sync-wait per
    instruction ("Too many sync wait commands" otherwise). Move extra
    waits onto same-engine NOPs inserted just before the instruction."""
    k = 0
    for bb in nc.main_func.blocks:
        out = []
        dirty = False
        for inst in bb.instructions:
            si = inst.sync_info
            if si is not None and len(si.on_wait) > limit:
                ow = list(si.on_wait)
                keep, extra = ow[-limit:], ow[:-limit]
                del si.on_wait[:]
                for w in keep:
                    si.on_wait.append(w)
                for i in range(0, len(extra), limit):
                    k += 1
                    out.append(
                        mybir.InstNoOp(
                            name=f"WSPLIT-{k}",
                            engine=inst.engine,
                            sync_info=mybir.SyncInfo(
                                on_wait=list(extra[i : i + limit]), on_update=[]
                            ),
                            text_hint="wsplit",
                            bass_nofuse=True,
                        )
                    )
                dirty = True
            out.append(inst)
        if dirty:
            bb.instructions[:] = out
    return k


def _mdview(ap):
    return ap.rearrange("p (m d) -> p m d", d=D)


def _build_squash_ln(nc, pool, vsum, gam_d, bet_d, v_d):
    """vsum [128, MD] f32 (raw summed v). Returns vs tile (squashed)."""
    tmp = pool.tile([128, MD], F32, tag="lntmp")
    nc.vector.tensor_mul(tmp[:], vsum[:], vsum[:])
    sq = pool.tile([128, M], F32, tag="sq")
    nc.vector.tensor_reduce(
        sq[:], _mdview(tmp[:]), axis=mybir.AxisListType.X, op=mybir.AluOpType.add
    )
    rt = pool.tile([128, M], F32, tag="rt")
    nc.scalar.activation(out=rt[:], in_=sq[:], func=mybir.ActivationFunctionType.Sqrt)
    den = pool.tile([128, M], F32, tag="den")
    nc.scalar.add(out=den[:], in_=sq[:], add=1.0)
    nc.vector.reciprocal(den[:], den[:])
    fac = pool.tile([128, M], F32, tag="fac")
    nc.vector.tensor_mul(fac[:], rt[:], den[:])
    vs = pool.tile([128, MD], F32, tag="vs")
    nc.vector.tensor_mul(
        _mdview(vs[:]), _mdview(vsum[:]), fac[:, :, None].broadcast_to([128, M, D])
    )

    # LayerNorm over d
    mu = pool.tile([128, M], F32, tag="mu")
    nc.vector.tensor_reduce(
        mu[:], _mdview(vs[:]), axis=mybir.AxisListType.X, op=mybir.AluOpType.add
    )
    nc.scalar.mul(out=mu[:], in_=mu[:], mul=1.0 / D)
    cen = pool.tile([128, MD], F32, tag="cen")
    nc.vector.tensor_sub(
        _mdview(cen[:]), _mdview(vs[:]), mu[:, :, None].broadcast_to([128, M, D])
    )
    c2 = pool.tile([128, MD], F32, tag="lntmp")
    nc.vector.tensor_mul(c2[:], cen[:], cen[:])
    var = pool.tile([128, M], F32, tag="var")
    nc.vector.tensor_reduce(
        var[:], _mdview(c2[:]), axis=mybir.AxisListType.X, op=mybir.AluOpType.add
    )
    eps_t = pool.tile([128, 1], F32, tag="eps")
    nc.vector.memset(eps_t[:], LN_EPS)
    rstd = pool.tile([128, M], F32, tag="rstd")
    nc.scalar.activation(
        out=rstd[:],
        in_=var[:],
        func=mybir.ActivationFunctionType.Sqrt,
        bias=eps_t[:],
        scale=1.0 / D,
    )
    nc.vector.reciprocal(rstd[:], rstd[:])
    vout = pool.tile([128, MD], F32, tag="vout")
    nc.vector.tensor_mul(
        _mdview(vout[:]), _mdview(cen[:]), rstd[:, :, None].broadcast_to([128, M, D])
    )
    gb = pool.tile([128, 2, D], F32, tag="gb")
    nc.sync.dma_start(
        out=gb[:, 0, :],
        in_=bass.AP(
            tensor=gam_d.ap().tensor, offset=gam_d.ap().offset,
            ap=[[0, 128]] + [list(p) for p in gam_d.ap().ap],
        ),
    )
    nc.sync.dma_start(
        out=gb[:, 1, :],
        in_=bass.AP(
            tensor=bet_d.ap().tensor, offset=bet_d.ap().offset,
            ap=[[0, 128]] + [list(p) for p in bet_d.ap().ap],
        ),
    )
    nc.vector.tensor_mul(
        _mdview(vout[:]), _mdview(vout[:]),
        gb[:, 0, None, :].broadcast_to([128, M, D]),
    )
    nc.vector.tensor_add(
        _mdview(vout[:]), _mdview(vout[:]),
        gb[:, 1, None, :].broadcast_to([128, M, D]),
    )
    nc.sync.dma_start(out=v_d.ap(), in_=vout[:])
    return vs


def build_kernel(phase2=True):
    nc = bass.Bass()
    # xte/xto: x shard transposed to [na, b] with odd/even n_local rows
    # zeroed (host-prepared layout) — enables K=32 row-tile matmuls.
    xte_bf_d = nc.dram_tensor("xte_bf", [KS, B], BF16, kind="ExternalInput")
    xto_bf_d = nc.dram_tensor("xto_bf", [KS, B], BF16, kind="ExternalInput")
    w_bf_d = nc.dram_tensor("w_bf", [KS, MD], BF16, kind="ExternalInput")
    gam_d = nc.dram_tensor("gam", [D], F32, kind="ExternalInput")
    bet_d = nc.dram_tensor("bet", [D], F32, kind="ExternalInput")
    v_d = nc.dram_tensor("v", [B, MD], F32, kind="ExternalOutput")
    qk_d = nc.dram_tensor("qk", [B, NS, M], F32, kind="ExternalOutput")

    w_view = w_bf_d.rearrange("(j p) md -> p j md", p=128)

    with tile.TileContext(nc) as tc:
        with (
            tc.tile_pool(name="singles", bufs=1) as singles,
            tc.tile_pool(name="wstream", bufs=6) as w_pool,
            tc.tile_pool(name="dram", bufs=1, space="DRAM") as dram,
        ):
            xte_bf = singles.tile([128, NCHUNK, 128], BF16)
            xto_bf = singles.tile([128, NCHUNK, 128], BF16)
            xte_v = xte_bf_d.rearrange("(j p) b -> p j b", p=128)
            xto_v = xto_bf_d.rearrange("(j p) b -> p j b", p=128)
            for q in range(4):
                qs = slice(q * NCHUNK // 4, (q + 1) * NCHUNK // 4)
                nc.sync.dma_start(out=xte_bf[:, qs, :], in_=xte_v[:, qs, :])
                nc.sync.dma_start(out=xto_bf[:, qs, :], in_=xto_v[:, qs, :])

            with tc.tile_pool(name="psum_v", bufs=1, space="PSUM") as psum_v:
                # ---- phase 1: v_part = sum_j (XTe_j + XTo_j).T @ W_j ----
                vp = psum_v.tile([128, MD], F32)
                for j in range(NCHUNK):
                    wj = w_pool.tile([128, MD], BF16, tag="wbf")
                    nc.sync.dma_start(out=wj[:], in_=w_view[:, j, :])
                    for c in range(2):
                        for pi, xt in enumerate((xte_bf, xto_bf)):
                            nc.tensor.matmul(
                                vp[:, c * 512 : (c + 1) * 512],
                                xt[:, j, :],
                                wj[:, c * 512 : (c + 1) * 512],
                                start=(j == 0 and pi == 0),
                                stop=(j == NCHUNK - 1 and pi == 1),
                            )

                vpart = singles.tile([128, MD], F32)
                nc.scalar.mul(out=vpart[:], in_=vp[:], mul=1.0 / M)

            ar_in = dram.tile([128, MD], F32)
            ar_out = dram.tile([128, MD], F32)
            nc.sync.dma_start(out=ar_in[:], in_=vpart[:])
            nc.gpsimd.collective_compute(
                "AllReduce",
                mybir.AluOpType.add,
                replica_groups=[list(range(NCORES))],
                ins=[ar_in[:].opt()],
                outs=[ar_out[:].opt()],
            )
            vsum = singles.tile([128, MD], F32)
            nc.sync.dma_start(out=vsum[:], in_=ar_out[:])

            vs = _build_squash_ln(nc, singles, vsum, gam_d, bet_d, v_d)

            if phase2:
                _phase2(nc, tc, w_view, qk_d, xte_bf, xto_bf, vs, w_pool)

    _split_waits(nc)
    return nc


def _phase2(nc, tc, w_view, qk_d, xte, xto, vs, w_pool):
    with (
        tc.tile_pool(name="p2sb", bufs=1) as p2sb,
        tc.tile_pool(name="psum_u", bufs=2, space="PSUM") as psum_u,
        tc.tile_pool(name="ubf", bufs=10) as ubf_pool,
        tc.tile_pool(name="prod", bufs=4) as prod_pool,
        tc.tile_pool(name="tree", bufs=4) as tree_pool,
        tc.tile_pool(name="qkst", bufs=3) as qk_pool,
    ):
        vs_bf = p2sb.tile([128, MD], BF16)
        nc.vector.tensor_copy(out=vs_bf[:], in_=vs[:])

        treeidx = 0
        for j in range(NCHUNK):
            wj = w_pool.tile([128, MD], BF16, tag="wbf")
            nc.sync.dma_start(out=wj[:], in_=w_view[:, j, :])
            qk_st = qk_pool.tile([128, 8, M], F32)
            for par, xt in ((0, xte), (1, xto)):
                # prod [128, 4n, (c, 32m, 16d)] collects both md-halves so
                # one tree pass covers the whole parity group.
                prod = prod_pool.tile([128, 4, 2, 512], BF16)
                for c in range(2):
                    up = psum_u.tile([128, 4, 512], F32)
                    for i in range(4):
                        nc.tensor.matmul(
                            up[:, i, :],
                            xt[32 * i : 32 * i + 32, j, :],
                            wj[32 * i : 32 * i + 32, c * 512 : (c + 1) * 512],
                            start=True,
                            stop=True,
                            tile_position=(32 * i, 0),
                        )
                    # drain PSUM -> SBUF bf16 on ScalarE
                    ubf = ubf_pool.tile([128, 4, 512], BF16)
                    nc.scalar.copy(out=ubf[:], in_=up[:])
                    # multiply by vs (broadcast over the 4 capsules)
                    nc.vector.tensor_mul(
                        prod[:, :, c, :],
                        ubf[:],
                        vs_bf[:, None, c * 512 : (c + 1) * 512].broadcast_to(
                            [128, 4, 512]
                        ),
                    )
                # tree-reduce over d (16 -> 1); Pool takes half
                red_eng = nc.gpsimd if (treeidx % 2 == 0) else nc.vector
                treeidx += 1
                pv = prod[:].rearrange(
                    "p n c (m d8 two) -> p (n c m) d8 two", d8=8, two=2
                )
                t8 = tree_pool.tile([128, 256, 8], BF16, tag="t8")
                red_eng.tensor_add(t8[:], pv[:, :, :, 0], pv[:, :, :, 1])
                t8v = t8[:].rearrange("p q (d4 two) -> p q d4 two", two=2)
                t4 = tree_pool.tile([128, 256, 4], BF16, tag="t4")
                red_eng.tensor_add(t4[:], t8v[:, :, :, 0], t8v[:, :, :, 1])
                t4v = t4[:].rearrange("p q (d2 two) -> p q d2 two", two=2)
                t2 = tree_pool.tile([128, 256, 2], BF16, tag="t2")
                red_eng.tensor_add(t2[:], t4v[:, :, :, 0], t4v[:, :, :, 1])
                # final level -> fp32 into qk staging; q = (n4, c, m32) and
                # global m = c*32 + m32, so (c, m32) is the m axis in order
                qv = qk_st[:].rearrange("p (n4 two) m -> p n4 two m", two=2)
                t2v = t2[:].rearrange("p (n4 m) two -> p n4 m two", n4=4)
                red_eng.tensor_add(
                    qv[:, :, par, :], t2v[:, :, :, 0], t2v[:, :, :, 1]
                )
            nc.sync.dma_start(out=qk_d[:, 8 * j : 8 * j + 8, :], in_=qk_st[:])


_NC_CACHE = {}


def _get_nc(phase2=True):
    if phase2 not in _NC_CACHE:
        _NC_CACHE[phase2] = build_kernel(phase2)
    return _NC_CACHE[phase2]


def kernel(input, w, ln_gamma, ln_beta, _trace=False, _phase2=True):
    """Full inputs -> full outputs (new_qk [B,N,M], v_out [B,M,D])."""
    input = np.ascontiguousarray(np.asarray(input, dtype=np.float32))
    w = np.ascontiguousarray(np.asarray(w, dtype=np.float32))
    ln_gamma = np.ascontiguousarray(np.asarray(ln_gamma, dtype=np.float32))
    ln_beta = np.ascontiguousarray(np.asarray(ln_beta, dtype=np.float32))

    nc = _get_nc(_phase2)

    # parity masks over na-rows: n_local = (row//A) % 8; even/odd zeroed
    row_n = (np.arange(KS) // A) % 8
    even_rows = (row_n % 2 == 0)[:, None]

    import ml_dtypes

    bf16 = ml_dtypes.bfloat16
    w_bf_full = w.reshape(N * A, MD).astype(bf16)
    zero = bf16(0)

    in_maps = []
    for core in range(NCORES):
        n0 = core * NS
        xt = input[:, n0 : n0 + NS, :].reshape(B, KS).T.astype(bf16)
        in_maps.append(
            {
                "xte_bf": np.where(even_rows, xt, zero),
                "xto_bf": np.where(even_rows, zero, xt),
                "w_bf": w_bf_full[n0 * A : (n0 + NS) * A],
                "gam": ln_gamma,
                "bet": ln_beta,
            }
        )

    res = run_bass_kernel_spmd(nc, in_maps, list(range(NCORES)), trace=_trace)

    new_qk = np.empty((B, N, M), dtype=np.float32)
    for core in range(NCORES):
        n0 = core * NS
        new_qk[:, n0 : n0 + NS, :] = res.results[core]["qk"]
    v_out = res.results[0]["v"].reshape(B, M, D)
    if _trace:
        kernel._last_results = res
    return new_qk, v_out


# revision 40
# speedup vs baseline: 1.0052x; 1.0052x over previous
"""Trainium2 Bass kernel for nn_DRCapsuleFC_79817672229002 (8 NeuronCores).

Math (see reference):
  B=128, N=2048, A=16, M=64, D=16; qk = 1/M (uniform routing)
  v[b,m,d]    = (1/M) * sum_{n,a} x[b,n,a] * w[n,a,m,d]
  vs          = squash_d(v) = v * sqrt(|v|^2)/(1+|v|^2)
  new_qk[b,n,m] = sum_{a,d} x[b,n,a] * w[n,a,m,d] * vs[b,m,d]
  v_out       = LN_d(vs) * gamma + beta

Sharding: tensor-parallel over n (in_n_capsules): 8 cores x 256 n each.
Per core:
  - host prepares XTe/XTo = x shard transposed to [na, b] (bf16) with
    odd/even n_local rows zeroed (enables K=32 row-tile matmuls that
    compute per-capsule U_n without mixing neighbouring capsules).
  - phase 1: v_part = sum_j (XTe_j + XTo_j).T @ W_j  (w streamed, bf16)
    -> fp32 on-chip AllReduce (512KB) -> squash + LN (DVE/ACT)
  - phase 2: w streamed again; per 128-row chunk, parity and md-half,
    4 concurrent K=32 row-tile matmuls produce U_n[b, md] in PSUM;
    ScalarE drains PSUM->SBUF bf16, DVE multiplies by vs, DVE/GpSimd
    tree-reduce over d -> new_qk[b,n,m].

All matmuls run in bf16 (fp32 matmul is quarter-rate on the PE);
accumulation stays fp32 in PSUM. Walrus in this container accepts at
most ONE sync-wait per instruction, so _split_waits() spreads Tile's
multi-waits over same-engine NOPs after scheduling.
"""

import sys

sys.path.insert(0, "/opt/trn_rl_repo")

import numpy as np

import concourse.bass as bass
import concourse.mybir as mybir
import concourse.tile as tile
from concourse.bass_utils import run_bass_kernel_spmd

F32 = mybir.dt.float32
BF16 = mybir.dt.bfloat16

B = 128
N = 2048
A = 16
M = 64
D = 16
MD = M * D  # 1024
NCORES = 8
NS = N // NCORES  # 256
KS = NS * A  # 4096
NCHUNK = KS // 128  # 32
LN_EPS = 1e-5


def _split_waits(nc, limit=1):
    """Walrus in this container accepts at most one sync-wait per
    instruction ("Too many sync wait commands" otherwise). Move extra
    waits onto same-engine NOPs inserted just before the instruction."""
    k = 0
    for bb in nc.main_func.blocks:
        out = []
        dirty = False
        for inst in bb.instructions:
            si = inst.sync_info
            if si is not None and len(si.on_wait) > limit:
                ow = list(si.on_wait)
                keep, extra = ow[-limit:], ow[:-limit]
                del si.on_wait[:]
                for w in keep:
                    si.on_wait.append(w)
                for i in range(0, len(extra), limit):
                    k += 1
                    out.append(
                        mybir.InstNoOp(
                            name=f"WSPLIT-{k}",
                            engine=inst.engine,
                            sync_info=mybir.SyncInfo(
                                on_wait=list(extra[i : i + limit]), on_update=[]
                            ),
                            text_hint="wsplit",
                            bass_nofuse=True,
                        )
                    )
                dirty = True
            out.append(inst)
        if dirty:
            bb.instructions[:] = out
    return k


def _mdview(ap):
    return ap.rearrange("p (m d) -> p m d", d=D)


def _build_squash_ln(nc, pool, vsum, gam_d, bet_d, v_d):
    """vsum [128, MD] f32 (raw summed v). Returns vs tile (squashed)."""
    tmp = pool.tile([128, MD], F32, tag="lntmp")
    nc.vector.tensor_mul(tmp[:], vsum[:], vsum[:])
    sq = pool.tile([128, M], F32, tag="sq")
    nc.vector.tensor_reduce(
        sq[:], _mdview(tmp[:]), axis=mybir.AxisListType.X, op=mybir.AluOpType.add
    )
    rt = pool.tile([128, M], F32, tag="rt")
    nc.scalar.activation(out=rt[:], in_=sq[:], func=mybir.ActivationFunctionType.Sqrt)
    den = pool.tile([128, M], F32, tag="den")
    nc.scalar.add(out=den[:], in_=sq[:], add=1.0)
    nc.vector.reciprocal(den[:], den[:])
    fac = pool.tile([128, M], F32, tag="fac")
    nc.vector.tensor_mul(fac[:], rt[:], den[:])
    vs = pool.tile([128, MD], F32, tag="vs")
    nc.vector.tensor_mul(
        _mdview(vs[:]), _mdview(vsum[:]), fac[:, :, None].broadcast_to([128, M, D])
    )

    # LayerNorm over d
    mu = pool.tile([128, M], F32, tag="mu")
    nc.vector.tensor_reduce(
        mu[:], _mdview(vs[:]), axis=mybir.AxisListType.X, op=mybir.AluOpType.add
    )
    nc.scalar.mul(out=mu[:], in_=mu[:], mul=1.0 / D)
    cen = pool.tile([128, MD], F32, tag="cen")
    nc.vector.tensor_sub(
        _mdview(cen[:]), _mdview(vs[:]), mu[:, :, None].broadcast_to([128, M, D])
    )
    c2 = pool.tile([128, MD], F32, tag="lntmp")
    nc.vector.tensor_mul(c2[:], cen[:], cen[:])
    var = pool.tile([128, M], F32, tag="var")
    nc.vector.tensor_reduce(
        var[:], _mdview(c2[:]), axis=mybir.AxisListType.X, op=mybir.AluOpType.add
    )
    eps_t = pool.tile([128, 1], F32, tag="eps")
    nc.vector.memset(eps_t[:], LN_EPS)
    rstd = pool.tile([128, M], F32, tag="rstd")
    nc.scalar.activation(
        out=rstd[:],
        in_=var[:],
        func=mybir.ActivationFunctionType.Sqrt,
        bias=eps_t[:],
        scale=1.0 / D,
    )
    nc.vector.reciprocal(rstd[:], rstd[:])
    vout = pool.tile([128, MD], F32, tag="vout")
    nc.vector.tensor_mul(
        _mdview(vout[:]), _mdview(cen[:]), rstd[:, :, None].broadcast_to([128, M, D])
    )
    gb = pool.tile([128, 2, D], F32, tag="gb")
    nc.sync.dma_start(
        out=gb[:, 0, :],
        in_=bass.AP(
            tensor=gam_d.ap().tensor, offset=gam_d.ap().offset,
            ap=[[0, 128]] + [list(p) for p in gam_d.ap().ap],
        ),
    )
    nc.sync.dma_start(
        out=gb[:, 1, :],
        in_=bass.AP(
            tensor=bet_d.ap().tensor, offset=bet_d.ap().offset,
            ap=[[0, 128]] + [list(p) for p in bet_d.ap().ap],
        ),
    )
    nc.vector.tensor_mul(
        _mdview(vout[:]), _mdview(vout[:]),
        gb[:, 0, None, :].broadcast_to([128, M, D]),
    )
    nc.vector.tensor_add(
        _mdview(vout[:]), _mdview(vout[:]),
        gb[:, 1, None, :].broadcast_to([128, M, D]),
    )
    nc.sync.dma_start(out=v_d.ap(), in_=vout[:])
    return vs


def build_kernel(phase2=True):
    nc = bass.Bass()
    # xte/xto: x shard transposed to [na, b] with odd/even n_local rows
    # zeroed (host-prepared layout) — enables K=32 row-tile matmuls.
    xte_bf_d = nc.dram_tensor("xte_bf", [KS, B], BF16, kind="ExternalInput")
    xto_bf_d = nc.dram_tensor("xto_bf", [KS, B], BF16, kind="ExternalInput")
    w_bf_d = nc.dram_tensor("w_bf", [KS, MD], BF16, kind="ExternalInput")
    gam_d = nc.dram_tensor("gam", [D], F32, kind="ExternalInput")
    bet_d = nc.dram_tensor("bet", [D], F32, kind="ExternalInput")
    v_d = nc.dram_tensor("v", [B, MD], F32, kind="ExternalOutput")
    qk_d = nc.dram_tensor("qk", [B, NS, M], F32, kind="ExternalOutput")

    w_view = w_bf_d.rearrange("(j p) md -> p j md", p=128)

    with tile.TileContext(nc) as tc:
        with (
            tc.tile_pool(name="singles", bufs=1) as singles,
            tc.tile_pool(name="wstream", bufs=6) as w_pool,
            tc.tile_pool(name="dram", bufs=1, space="DRAM") as dram,
        ):
            xte_bf = singles.tile([128, NCHUNK, 128], BF16)
            xto_bf = singles.tile([128, NCHUNK, 128], BF16)
            xte_v = xte_bf_d.rearrange("(j p) b -> p j b", p=128)
            xto_v = xto_bf_d.rearrange("(j p) b -> p j b", p=128)
            for q in range(4):
                qs = slice(q * NCHUNK // 4, (q + 1) * NCHUNK // 4)
                nc.sync.dma_start(out=xte_bf[:, qs, :], in_=xte_v[:, qs, :])
                nc.sync.dma_start(out=xto_bf[:, qs, :], in_=xto_v[:, qs, :])

            with tc.tile_pool(name="psum_v", bufs=1, space="PSUM") as psum_v:
                # ---- phase 1: v_part = sum_j (XTe_j + XTo_j).T @ W_j ----
                # high_priority pulls the AR dependency chain forward so
                # phase-2 consumption unblocks as early as possible.
                with tc.high_priority():
                    vp = psum_v.tile([128, MD], F32)
                    for j in range(NCHUNK):
                        wj = w_pool.tile([128, MD], BF16, tag="wbf")
                        nc.sync.dma_start(out=wj[:], in_=w_view[:, j, :])
                        for c in range(2):
                            for pi, xt in enumerate((xte_bf, xto_bf)):
                                nc.tensor.matmul(
                                    vp[:, c * 512 : (c + 1) * 512],
                                    xt[:, j, :],
                                    wj[:, c * 512 : (c + 1) * 512],
                                    start=(j == 0 and pi == 0),
                                    stop=(j == NCHUNK - 1 and pi == 1),
                                )

                    vpart = singles.tile([128, MD], F32)
                    nc.scalar.mul(out=vpart[:], in_=vp[:], mul=1.0 / M)

            with tc.high_priority():
                ar_in = dram.tile([128, MD], F32)
                ar_out = dram.tile([128, MD], F32)
                nc.sync.dma_start(out=ar_in[:], in_=vpart[:])
                nc.gpsimd.collective_compute(
                    "AllReduce",
                    mybir.AluOpType.add,
                    replica_groups=[list(range(NCORES))],
                    ins=[ar_in[:].opt()],
                    outs=[ar_out[:].opt()],
                )
                vsum = singles.tile([128, MD], F32)
                nc.sync.dma_start(out=vsum[:], in_=ar_out[:])

            vs = _build_squash_ln(nc, singles, vsum, gam_d, bet_d, v_d)

            if phase2:
                _phase2(nc, tc, w_view, qk_d, xte_bf, xto_bf, vs, w_pool)

    _split_waits(nc)
    return nc


def _phase2(nc, tc, w_view, qk_d, xte, xto, vs, w_pool):
    with (
        tc.tile_pool(name="p2sb", bufs=1) as p2sb,
        tc.tile_pool(name="psum_u", bufs=2, space="PSUM") as psum_u,
        tc.tile_pool(name="ubf", bufs=10) as ubf_pool,
        tc.tile_pool(name="prod", bufs=4) as prod_pool,
        tc.tile_pool(name="tree", bufs=4) as tree_pool,
        tc.tile_pool(name="qkst", bufs=3) as qk_pool,
    ):
        vs_bf = p2sb.tile([128, MD], BF16)
        nc.vector.tensor_copy(out=vs_bf[:], in_=vs[:])

        treeidx = 0
        for j in range(NCHUNK):
            wj = w_pool.tile([128, MD], BF16, tag="wbf")
            nc.sync.dma_start(out=wj[:], in_=w_view[:, j, :])
            qk_st = qk_pool.tile([128, 8, M], F32)
            for par, xt in ((0, xte), (1, xto)):
                # prod [128, 4n, (c, 32m, 16d)] collects both md-halves so
                # one tree pass covers the whole parity group.
                prod = prod_pool.tile([128, 4, 2, 512], BF16)
                for c in range(2):
                    up = psum_u.tile([128, 4, 512], F32)
                    for i in range(4):
                        nc.tensor.matmul(
                            up[:, i, :],
                            xt[32 * i : 32 * i + 32, j, :],
                            wj[32 * i : 32 * i + 32, c * 512 : (c + 1) * 512],
                            start=True,
                            stop=True,
                            tile_position=(32 * i, 0),
                        )
                    # drain PSUM -> SBUF bf16 on ScalarE
                    ubf = ubf_pool.tile([128, 4, 512], BF16)
                    nc.scalar.copy(out=ubf[:], in_=up[:])
                    # multiply by vs (broadcast over the 4 capsules)
                    nc.vector.tensor_mul(
                        prod[:, :, c, :],
                        ubf[:],
                        vs_bf[:, None, c * 512 : (c + 1) * 512].broadcast_to(
                            [128, 4, 512]
                        ),
                    )
                # tree-reduce over d (16 -> 1); Pool takes half
                red_eng = nc.gpsimd if (treeidx % 2 == 0) else nc.vector
                treeidx += 1
                pv = prod[:].rearrange(
                    "p n c (m d8 two) -> p (n c m) d8 two", d8=8, two=2
                )
                t8 = tree_pool.tile([128, 256, 8], BF16, tag="t8")
                red_eng.tensor_add(t8[:], pv[:, :, :, 0], pv[:, :, :, 1])
                t8v = t8[:].rearrange("p q (d4 two) -> p q d4 two", two=2)
                t4 = tree_pool.tile([128, 256, 4], BF16, tag="t4")
                red_eng.tensor_add(t4[:], t8v[:, :, :, 0], t8v[:, :, :, 1])
                t4v = t4[:].rearrange("p q (d2 two) -> p q d2 two", two=2)
                t2 = tree_pool.tile([128, 256, 2], BF16, tag="t2")
                red_eng.tensor_add(t2[:], t4v[:, :, :, 0], t4v[:, :, :, 1])
                # final level -> fp32 into qk staging; q = (n4, c, m32) and
                # global m = c*32 + m32, so (c, m32) is the m axis in order
                qv = qk_st[:].rearrange("p (n4 two) m -> p n4 two m", two=2)
                t2v = t2[:].rearrange("p (n4 m) two -> p n4 m two", n4=4)
                red_eng.tensor_add(
                    qv[:, :, par, :], t2v[:, :, :, 0], t2v[:, :, :, 1]
                )
            nc.sync.dma_start(out=qk_d[:, 8 * j : 8 * j + 8, :], in_=qk_st[:])


_NC_CACHE = {}


def _get_nc(phase2=True):
    if phase2 not in _NC_CACHE:
        _NC_CACHE[phase2] = build_kernel(phase2)
    return _NC_CACHE[phase2]


def kernel(input, w, ln_gamma, ln_beta, _trace=False, _phase2=True):
    """Full inputs -> full outputs (new_qk [B,N,M], v_out [B,M,D])."""
    input = np.ascontiguousarray(np.asarray(input, dtype=np.float32))
    w = np.ascontiguousarray(np.asarray(w, dtype=np.float32))
    ln_gamma = np.ascontiguousarray(np.asarray(ln_gamma, dtype=np.float32))
    ln_beta = np.ascontiguousarray(np.asarray(ln_beta, dtype=np.float32))

    nc = _get_nc(_phase2)

    # parity masks over na-rows: n_local = (row//A) % 8; even/odd zeroed
    row_n = (np.arange(KS) // A) % 8
    even_rows = (row_n % 2 == 0)[:, None]

    import ml_dtypes

    bf16 = ml_dtypes.bfloat16
    w_bf_full = w.reshape(N * A, MD).astype(bf16)
    zero = bf16(0)

    in_maps = []
    for core in range(NCORES):
        n0 = core * NS
        xt = input[:, n0 : n0 + NS, :].reshape(B, KS).T.astype(bf16)
        in_maps.append(
            {
                "xte_bf": np.where(even_rows, xt, zero),
                "xto_bf": np.where(even_rows, zero, xt),
                "w_bf": w_bf_full[n0 * A : (n0 + NS) * A],
                "gam": ln_gamma,
                "bet": ln_beta,
            }
        )

    res = run_bass_kernel_spmd(nc, in_maps, list(range(NCORES)), trace=_trace)

    new_qk = np.empty((B, N, M), dtype=np.float32)
    for core in range(NCORES):
        n0 = core * NS
        new_qk[:, n0 : n0 + NS, :] = res.results[core]["qk"]
    v_out = res.results[0]["v"].reshape(B, M, D)
    if _trace:
        kernel._last_results = res
    return new_qk, v_out


# revision 45
# speedup vs baseline: 1.0116x; 1.0064x over previous
"""Trainium2 Bass kernel for nn_DRCapsuleFC_79817672229002 (8 NeuronCores).

Math (see reference):
  B=128, N=2048, A=16, M=64, D=16; qk = 1/M (uniform routing)
  v[b,m,d]    = (1/M) * sum_{n,a} x[b,n,a] * w[n,a,m,d]
  vs          = squash_d(v) = v * sqrt(|v|^2)/(1+|v|^2)
  new_qk[b,n,m] = sum_{a,d} x[b,n,a] * w[n,a,m,d] * vs[b,m,d]
  v_out       = LN_d(vs) * gamma + beta

Sharding: tensor-parallel over n (in_n_capsules): 8 cores x 256 n each.
Per core:
  - host prepares XTe/XTo = x shard transposed to [na, b] (bf16) with
    odd/even n_local rows zeroed (enables K=32 row-tile matmuls that
    compute per-capsule U_n without mixing neighbouring capsules).
  - phase 1: v_part = sum_j (XTe_j + XTo_j).T @ W_j  (w streamed, bf16)
    -> fp32 on-chip AllReduce (512KB) -> squash + LN (DVE/ACT)
  - phase 2: w streamed again; per 128-row chunk, parity and md-half,
    4 concurrent K=32 row-tile matmuls produce U_n[b, md] in PSUM;
    ScalarE drains PSUM->SBUF bf16, DVE multiplies by vs, DVE/GpSimd
    tree-reduce over d -> new_qk[b,n,m].

All matmuls run in bf16 (fp32 matmul is quarter-rate on the PE);
accumulation stays fp32 in PSUM. Walrus in this container accepts at
most ONE sync-wait per instruction, so _split_waits() spreads Tile's
multi-waits over same-engine NOPs after scheduling.
"""

import sys

sys.path.insert(0, "/opt/trn_rl_repo")

import numpy as np

import concourse.bass as bass
import concourse.mybir as mybir
import concourse.tile as tile
from concourse.bass_utils import run_bass_kernel_spmd

F32 = mybir.dt.float32
BF16 = mybir.dt.bfloat16

B = 128
N = 2048
A = 16
M = 64
D = 16
MD = M * D  # 1024
NCORES = 8
NS = N // NCORES  # 256
KS = NS * A  # 4096
NCHUNK = KS // 128  # 32
LN_EPS = 1e-5


def _split_waits(nc, limit=1):
    """Walrus in this container accepts at most one sync-wait per
    instruction ("Too many sync wait commands" otherwise). Move extra
    waits onto same-engine NOPs inserted just before the instruction."""
    k = 0
    for bb in nc.main_func.blocks:
        out = []
        dirty = False
        for inst in bb.instructions:
            si = inst.sync_info
            if si is not None and len(si.on_wait) > limit:
                ow = list(si.on_wait)
                keep, extra = ow[-limit:], ow[:-limit]
                del si.on_wait[:]
                for w in keep:
                    si.on_wait.append(w)
                for i in range(0, len(extra), limit):
                    k += 1
                    out.append(
                        mybir.InstNoOp(
                            name=f"WSPLIT-{k}",
                            engine=inst.engine,
                            sync_info=mybir.SyncInfo(
                                on_wait=list(extra[i : i + limit]), on_update=[]
                            ),
                            text_hint="wsplit",
                            bass_nofuse=True,
                        )
                    )
                dirty = True
            out.append(inst)
        if dirty:
            bb.instructions[:] = out
    return k


def _mdview(ap):
    return ap.rearrange("p (m d) -> p m d", d=D)


def _build_squash_ln(nc, pool, vsum, gam_d, bet_d, v_d):
    """vsum [128, MD] f32 (raw summed v). Returns vs tile (squashed)."""
    tmp = pool.tile([128, MD], F32, tag="lntmp")
    nc.vector.tensor_mul(tmp[:], vsum[:], vsum[:])
    sq = pool.tile([128, M], F32, tag="sq")
    nc.vector.tensor_reduce(
        sq[:], _mdview(tmp[:]), axis=mybir.AxisListType.X, op=mybir.AluOpType.add
    )
    rt = pool.tile([128, M], F32, tag="rt")
    nc.scalar.activation(out=rt[:], in_=sq[:], func=mybir.ActivationFunctionType.Sqrt)
    den = pool.tile([128, M], F32, tag="den")
    nc.scalar.add(out=den[:], in_=sq[:], add=1.0)
    nc.vector.reciprocal(den[:], den[:])
    fac = pool.tile([128, M], F32, tag="fac")
    nc.vector.tensor_mul(fac[:], rt[:], den[:])
    vs = pool.tile([128, MD], F32, tag="vs")
    nc.vector.tensor_mul(
        _mdview(vs[:]), _mdview(vsum[:]), fac[:, :, None].broadcast_to([128, M, D])
    )

    # LayerNorm over d
    mu = pool.tile([128, M], F32, tag="mu")
    nc.vector.tensor_reduce(
        mu[:], _mdview(vs[:]), axis=mybir.AxisListType.X, op=mybir.AluOpType.add
    )
    nc.scalar.mul(out=mu[:], in_=mu[:], mul=1.0 / D)
    cen = pool.tile([128, MD], F32, tag="cen")
    nc.vector.tensor_sub(
        _mdview(cen[:]), _mdview(vs[:]), mu[:, :, None].broadcast_to([128, M, D])
    )
    c2 = pool.tile([128, MD], F32, tag="lntmp")
    nc.vector.tensor_mul(c2[:], cen[:], cen[:])
    var = pool.tile([128, M], F32, tag="var")
    nc.vector.tensor_reduce(
        var[:], _mdview(c2[:]), axis=mybir.AxisListType.X, op=mybir.AluOpType.add
    )
    eps_t = pool.tile([128, 1], F32, tag="eps")
    nc.vector.memset(eps_t[:], LN_EPS)
    rstd = pool.tile([128, M], F32, tag="rstd")
    nc.scalar.activation(
        out=rstd[:],
        in_=var[:],
        func=mybir.ActivationFunctionType.Sqrt,
        bias=eps_t[:],
        scale=1.0 / D,
    )
    nc.vector.reciprocal(rstd[:], rstd[:])
    vout = pool.tile([128, MD], F32, tag="vout")
    nc.vector.tensor_mul(
        _mdview(vout[:]), _mdview(cen[:]), rstd[:, :, None].broadcast_to([128, M, D])
    )
    gb = pool.tile([128, 2, D], F32, tag="gb")
    nc.sync.dma_start(
        out=gb[:, 0, :],
        in_=bass.AP(
            tensor=gam_d.ap().tensor, offset=gam_d.ap().offset,
            ap=[[0, 128]] + [list(p) for p in gam_d.ap().ap],
        ),
    )
    nc.sync.dma_start(
        out=gb[:, 1, :],
        in_=bass.AP(
            tensor=bet_d.ap().tensor, offset=bet_d.ap().offset,
            ap=[[0, 128]] + [list(p) for p in bet_d.ap().ap],
        ),
    )
    nc.vector.tensor_mul(
        _mdview(vout[:]), _mdview(vout[:]),
        gb[:, 0, None, :].broadcast_to([128, M, D]),
    )
    nc.vector.tensor_add(
        _mdview(vout[:]), _mdview(vout[:]),
        gb[:, 1, None, :].broadcast_to([128, M, D]),
    )
    nc.sync.dma_start(out=v_d.ap(), in_=vout[:])
    return vs


def build_kernel(phase2=True):
    nc = bass.Bass()
    # xte/xto: x shard transposed to [na, b] with odd/even n_local rows
    # zeroed (host-prepared layout) — enables K=32 row-tile matmuls.
    xte_bf_d = nc.dram_tensor("xte_bf", [KS, B], BF16, kind="ExternalInput")
    xto_bf_d = nc.dram_tensor("xto_bf", [KS, B], BF16, kind="ExternalInput")
    w_bf_d = nc.dram_tensor("w_bf", [KS, MD], BF16, kind="ExternalInput")
    gam_d = nc.dram_tensor("gam", [D], F32, kind="ExternalInput")
    bet_d = nc.dram_tensor("bet", [D], F32, kind="ExternalInput")
    v_d = nc.dram_tensor("v", [B, MD], F32, kind="ExternalOutput")
    qk_d = nc.dram_tensor("qk", [B, NS, M], F32, kind="ExternalOutput")

    w_view = w_bf_d.rearrange("(j p) md -> p j md", p=128)

    with tile.TileContext(nc) as tc:
        with (
            tc.tile_pool(name="singles", bufs=1) as singles,
            tc.tile_pool(name="wstream", bufs=6) as w_pool,
            tc.tile_pool(name="dram", bufs=1, space="DRAM") as dram,
        ):
            xte_bf = singles.tile([128, NCHUNK, 128], BF16)
            xto_bf = singles.tile([128, NCHUNK, 128], BF16)
            xte_v = xte_bf_d.rearrange("(j p) b -> p j b", p=128)
            xto_v = xto_bf_d.rearrange("(j p) b -> p j b", p=128)
            for q in range(4):
                qs = slice(q * NCHUNK // 4, (q + 1) * NCHUNK // 4)
                nc.sync.dma_start(out=xte_bf[:, qs, :], in_=xte_v[:, qs, :])
                nc.sync.dma_start(out=xto_bf[:, qs, :], in_=xto_v[:, qs, :])

            with tc.tile_pool(name="psum_v", bufs=1, space="PSUM") as psum_v:
                # ---- phase 1: v_part = sum_j (XTe_j + XTo_j).T @ W_j ----
                # high_priority pulls the AR dependency chain forward so
                # phase-2 consumption unblocks as early as possible.
                with tc.high_priority():
                    vp = psum_v.tile([128, MD], F32)
                    for j in range(NCHUNK):
                        wj = w_pool.tile([128, MD], BF16, tag="wbf")
                        nc.sync.dma_start(out=wj[:], in_=w_view[:, j, :])
                        for c in range(2):
                            for pi, xt in enumerate((xte_bf, xto_bf)):
                                nc.tensor.matmul(
                                    vp[:, c * 512 : (c + 1) * 512],
                                    xt[:, j, :],
                                    wj[:, c * 512 : (c + 1) * 512],
                                    start=(j == 0 and pi == 0),
                                    stop=(j == NCHUNK - 1 and pi == 1),
                                )

                    vpart = singles.tile([128, MD], F32)
                    nc.scalar.mul(out=vpart[:], in_=vp[:], mul=1.0 / M)

            with tc.high_priority():
                ar_in = dram.tile([128, MD], F32)
                ar_out = dram.tile([128, MD], F32)
                nc.sync.dma_start(out=ar_in[:], in_=vpart[:])
                nc.gpsimd.collective_compute(
                    "AllReduce",
                    mybir.AluOpType.add,
                    replica_groups=[list(range(NCORES))],
                    ins=[ar_in[:].opt()],
                    outs=[ar_out[:].opt()],
                )
                vsum = singles.tile([128, MD], F32)
                nc.sync.dma_start(out=vsum[:], in_=ar_out[:])

            vs = _build_squash_ln(nc, singles, vsum, gam_d, bet_d, v_d)

            if phase2:
                _phase2(nc, tc, w_view, qk_d, xte_bf, xto_bf, vs, w_pool)

    _split_waits(nc)
    return nc


def _phase2(nc, tc, w_view, qk_d, xte, xto, vs, w_pool):
    with (
        tc.tile_pool(name="p2sb", bufs=1) as p2sb,
        tc.tile_pool(name="psum_u", bufs=2, space="PSUM") as psum_u,
        tc.tile_pool(name="ubf", bufs=5) as ubf_pool,
        tc.tile_pool(name="prod", bufs=4) as prod_pool,
        tc.tile_pool(name="tree", bufs=4) as tree_pool,
        tc.tile_pool(name="qkst", bufs=3) as qk_pool,
    ):
        vs_bf = p2sb.tile([128, MD], BF16)
        nc.vector.tensor_copy(out=vs_bf[:], in_=vs[:])

        treeidx = 0
        for j in range(NCHUNK):
            wj = w_pool.tile([128, MD], BF16, tag="wbf")
            nc.sync.dma_start(out=wj[:], in_=w_view[:, j, :])
            qk_st = qk_pool.tile([128, 8, M], F32)
            for par, xt in ((0, xte), (1, xto)):
                # prod [128, 4n, (c, 32m, 16d)] collects both md-halves so
                # one tree pass covers the whole parity group.
                prod = prod_pool.tile([128, 4, 2, 512], BF16)
                ubf = ubf_pool.tile([128, 2, 4, 512], BF16)
                for c in range(2):
                    up = psum_u.tile([128, 4, 512], F32)
                    for i in range(4):
                        nc.tensor.matmul(
                            up[:, i, :],
                            xt[32 * i : 32 * i + 32, j, :],
                            wj[32 * i : 32 * i + 32, c * 512 : (c + 1) * 512],
                            start=True,
                            stop=True,
                            tile_position=(32 * i, 0),
                        )
                    # drain PSUM -> SBUF bf16 on ScalarE
                    nc.scalar.copy(out=ubf[:, c, :, :], in_=up[:])
                # single multiply by vs for both md-halves
                nc.vector.tensor_mul(
                    prod[:],
                    ubf[:].rearrange("p c n f -> p n c f"),
                    vs_bf[:, None, :].rearrange(
                        "p n (c f) -> p n c f", c=2
                    ).broadcast_to([128, 4, 2, 512]),
                )
                # tree-reduce over d (16 -> 1); Pool takes half
                red_eng = nc.gpsimd if (treeidx % 2 == 0) else nc.vector
                treeidx += 1
                pv = prod[:].rearrange(
                    "p n c (m d8 two) -> p (n c m) d8 two", d8=8, two=2
                )
                t8 = tree_pool.tile([128, 256, 8], BF16, tag="t8")
                red_eng.tensor_add(t8[:], pv[:, :, :, 0], pv[:, :, :, 1])
                t8v = t8[:].rearrange("p q (d4 two) -> p q d4 two", two=2)
                t4 = tree_pool.tile([128, 256, 4], BF16, tag="t4")
                red_eng.tensor_add(t4[:], t8v[:, :, :, 0], t8v[:, :, :, 1])
                t4v = t4[:].rearrange("p q (d2 two) -> p q d2 two", two=2)
                t2 = tree_pool.tile([128, 256, 2], BF16, tag="t2")
                red_eng.tensor_add(t2[:], t4v[:, :, :, 0], t4v[:, :, :, 1])
                # final level -> fp32 into qk staging; q = (n4, c, m32) and
                # global m = c*32 + m32, so (c, m32) is the m axis in order
                qv = qk_st[:].rearrange("p (n4 two) m -> p n4 two m", two=2)
                t2v = t2[:].rearrange("p (n4 m) two -> p n4 m two", n4=4)
                red_eng.tensor_add(
                    qv[:, :, par, :], t2v[:, :, :, 0], t2v[:, :, :, 1]
                )
            nc.sync.dma_start(out=qk_d[:, 8 * j : 8 * j + 8, :], in_=qk_st[:])


_NC_CACHE = {}


def _get_nc(phase2=True):
    if phase2 not in _NC_CACHE:
        _NC_CACHE[phase2] = build_kernel(phase2)
    return _NC_CACHE[phase2]


def kernel(input, w, ln_gamma, ln_beta, _trace=False, _phase2=True):
    """Full inputs -> full outputs (new_qk [B,N,M], v_out [B,M,D])."""
    input = np.ascontiguousarray(np.asarray(input, dtype=np.float32))
    w = np.ascontiguousarray(np.asarray(w, dtype=np.float32))
    ln_gamma = np.ascontiguousarray(np.asarray(ln_gamma, dtype=np.float32))
    ln_beta = np.ascontiguousarray(np.asarray(ln_beta, dtype=np.float32))

    nc = _get_nc(_phase2)

    # parity masks over na-rows: n_local = (row//A) % 8; even/odd zeroed
    row_n = (np.arange(KS) // A) % 8
    even_rows = (row_n % 2 == 0)[:, None]

    import ml_dtypes

    bf16 = ml_dtypes.bfloat16
    w_bf_full = w.reshape(N * A, MD).astype(bf16)
    zero = bf16(0)

    in_maps = []
    for core in range(NCORES):
        n0 = core * NS
        xt = input[:, n0 : n0 + NS, :].reshape(B, KS).T.astype(bf16)
        in_maps.append(
            {
                "xte_bf": np.where(even_rows, xt, zero),
                "xto_bf": np.where(even_rows, zero, xt),
                "w_bf": w_bf_full[n0 * A : (n0 + NS) * A],
                "gam": ln_gamma,
                "bet": ln_beta,
            }
        )

    res = run_bass_kernel_spmd(nc, in_maps, list(range(NCORES)), trace=_trace)

    new_qk = np.empty((B, N, M), dtype=np.float32)
    for core in range(NCORES):
        n0 = core * NS
        new_qk[:, n0 : n0 + NS, :] = res.results[core]["qk"]
    v_out = res.results[0]["v"].reshape(B, M, D)
    if _trace:
        kernel._last_results = res
    return new_qk, v_out
